# revision 1
# baseline (speedup 1.0000x reference)
"""Trainium2 Bass kernel for DecisionTreeModule forward.

Computes, for x [B, 256]: a 12-level complete-binary-tree traversal
(per-sample feature compares) followed by softmax(leaf_probabilities[leaf]).

Strategy (8 NeuronCores, pure data parallel over the batch):
  - Each core gets a padded shard of rows. Samples live one-per-partition in
    groups of G tiles ([128, G, 256] SBUF tiles).
  - Levels 0-6: node (feat, thr) looked up from per-level replicated SBUF
    tables by one-hot mask + segmented reduce (exact f32 select).
  - Levels 7-11: one indirect-DMA fetch per tile pulls a 62-float record
    (the (feat, thr) pairs of the whole 5-level subtree under the sample's
    level-7 node); within-record selects are narrow (<=16 wide).
  - The x-value select per level is a 256-wide one-hot mask + segmented
    reduce against the resident x tile (exact: 1.0*x + zeros).
  - Output: softmax table [4096, 100] built once on device from
    leaf_probabilities; per-tile indirect DMA gathers out[p] = smx[leaf[p]].

All compares are exact f32, so leaf indices match the reference exactly;
only the softmax arithmetic carries rounding error.
"""
import sys
sys.path.insert(0, "/opt/trn_rl_repo")

import numpy as np
import concourse.bacc as bacc
import concourse.bass as bass
import concourse.mybir as mybir
import concourse.tile as tile
from concourse.bass_utils import run_bass_kernel_spmd

P = 128
INPUT_DIM = 256
N_CLASSES = 100
MAX_DEPTH = 12
N_NODES = 2 ** MAX_DEPTH - 1     # 4095
N_LEAVES = 2 ** MAX_DEPTH        # 4096
NCORES = 8
REC_W = 62                       # 31 (feat, thr) pairs: levels 7..11 subtree

F32 = mybir.dt.float32
I32 = mybir.dt.int32
Alu = mybir.AluOpType


def _build_program(G: int, NG: int, f0: float, t0: float, repeat: int = 1):
    """Build the per-core Bass program. S = 128*G*NG samples."""
    S = P * G * NG
    nc = bacc.Bacc("TRN2", target_bir_lowering=False, debug=False)

    x = nc.dram_tensor("x", [S, INPUT_DIM], F32, kind="ExternalInput")
    lp = nc.dram_tensor("lp", [N_LEAVES, N_CLASSES], F32, kind="ExternalInput")
    iota = nc.dram_tensor("iota", [P, INPUT_DIM], F32, kind="ExternalInput")
    iotab = nc.dram_tensor("iotab", [P, INPUT_DIM], mybir.dt.bfloat16, kind="ExternalInput")
    # per-level (feat, thr) tables for levels 1..6, replicated per partition
    ftlev = {
        d: nc.dram_tensor(f"ft{d}", [P, 2, 2 ** d], F32, kind="ExternalInput")
        for d in range(1, 7)
    }
    rectab = nc.dram_tensor("rectab", [P, REC_W], F32, kind="ExternalInput")
    out = nc.dram_tensor("out", [S, N_CLASSES], F32, kind="ExternalOutput")
    smx = nc.dram_tensor("smx", [N_LEAVES, N_CLASSES], F32, kind="Internal")

    xg_all = x[:, :].rearrange("(g t p) f -> p g t f", p=P, t=G)
    og_all = out[:, :].rearrange("(g t p) c -> p g t c", p=P, t=G)
    lp_r = lp[:, :].rearrange("(p c) k -> p c k", p=P)
    smx_r = smx[:, :].rearrange("(p c) k -> p c k", p=P)

    with tile.TileContext(nc) as tc:
        with tc.tile_pool(name="cns", bufs=1) as cpool, \
             tc.tile_pool(name="xg", bufs=2) as xpool, \
             tc.tile_pool(name="mask", bufs=1) as mpool, \
             tc.tile_pool(name="xm", bufs=2) as xmpool, \
             tc.tile_pool(name="xp", bufs=2) as xppool, \
             tc.tile_pool(name="prod", bufs=1) as ppool, \
             tc.tile_pool(name="sml", bufs=3) as spool, \
             tc.tile_pool(name="rec", bufs=2) as rpool, \
             tc.tile_pool(name="orow", bufs=2) as opool:

            # ---- constants into SBUF ----
            t_iota = cpool.tile([P, 1, INPUT_DIM], F32)
            nc.sync.dma_start(t_iota[:], iota[:, :].rearrange("p (o f) -> p o f", o=1))
            t_iotab = cpool.tile([P, 1, INPUT_DIM], mybir.dt.bfloat16)
            nc.sync.dma_start(t_iotab[:], iotab[:, :].rearrange("p (o f) -> p o f", o=1))
            t_ft = {}
            for d in range(1, 7):
                t_ftd = cpool.tile([P, 1, 2, 2 ** d], F32, tag=f"ft{d}")
                nc.sync.dma_start(t_ftd[:], ftlev[d][:, :, :].rearrange("(p o) c w -> p o c w", o=1))
                t_ft[d] = t_ftd

            # ---- Part 1: softmax table smx = softmax(lp, axis=1) ----
            with tc.tile_pool(name="p1", bufs=1) as p1pool:
                t_lp = p1pool.tile([P, 32, N_CLASSES], F32)
                nc.sync.dma_start(t_lp[:], lp_r[:, :, :])
                t_exp = p1pool.tile([P, 32, N_CLASSES], F32)
                nc.scalar.activation(out=t_exp[:], in_=t_lp[:],
                                     func=mybir.ActivationFunctionType.Exp)
                t_sum = p1pool.tile([P, 32, 1], F32)
                nc.vector.tensor_reduce(t_sum[:], t_exp[:], mybir.AxisListType.X, Alu.add)
                t_rcp = p1pool.tile([P, 32, 1], F32)
                nc.vector.reciprocal(t_rcp[:], t_sum[:])
                nc.vector.tensor_tensor(
                    out=t_exp[:], in0=t_exp[:],
                    in1=t_rcp[:, :, :].to_broadcast([P, 32, N_CLASSES]),
                    op=Alu.mult)
                nc.sync.dma_start(smx_r[:, :, :], t_exp[:])

            # ---- Part 2: traversal per group ----
            rep_ctx = tc.For_i(0, repeat, 1) if repeat > 1 else None
            if rep_ctx is not None:
                rep_ctx.__enter__()
            for g in range(NG):
                t_x = xpool.tile([P, G, INPUT_DIM], F32, tag="x")
                nc.sync.dma_start(t_x[:], xg_all[:, g])

                node = None    # [P, G] f32, level-local node idx (levels 0-6)
                lnode = None   # [P, G] f32, subtree-local (levels 7-11)
                node7 = None
                t_rec = None

                for d in range(MAX_DEPTH):
                    # --- (feat, thr) for this level -> ft [P, G, 2] (or imm) ---
                    ft = None
                    if d == 0:
                        pass  # immediates f0, t0
                    elif d <= 6:
                        W = 2 ** d
                        t_nm = mpool.tile([P, G, 64], F32, tag="nmask")
                        nm = t_nm[:, :, :W]
                        nc.vector.tensor_tensor(
                            out=nm, in0=t_iota[:, :, :W].to_broadcast([P, G, W]),
                            in1=node[:, :, :].to_broadcast([P, G, W]),
                            op=Alu.is_equal)
                        t_pr = ppool.tile([P, G, 2, 64], F32, tag="nprod")
                        pr = t_pr[:, :, :, :W]
                        nc.vector.tensor_tensor(
                            out=pr,
                            in0=t_nm[:, :, :W].rearrange("p g (o w) -> p g o w", o=1).to_broadcast([P, G, 2, W]),
                            in1=t_ft[d][:, :, :, :W].to_broadcast([P, G, 2, W]),
                            op=Alu.mult)
                        ft = spool.tile([P, G, 2], F32, tag="ft")
                        nc.vector.tensor_reduce(ft[:], pr, mybir.AxisListType.X, Alu.add)
                    elif d == 7:
                        ft = t_rec[:, :, 0:2]
                    else:
                        j = d - 7
                        W = 2 ** j
                        base = 2 * (W - 1)
                        t_lm = mpool.tile([P, G, 16], F32, tag="lmask")
                        lm = t_lm[:, :, :W]
                        nc.vector.tensor_tensor(
                            out=lm, in0=t_iota[:, :, :W].to_broadcast([P, G, W]),
                            in1=lnode[:, :, :].to_broadcast([P, G, W]),
                            op=Alu.is_equal)
                        # record view [P, G, 2, W]: elem (c, l) at base + 2l + c
                        rv = t_rec[:, :, base:base + 2 * W].rearrange(
                            "p g (l c) -> p g c l", c=2)
                        t_pr = ppool.tile([P, G, 2, 16], F32, tag="lprod")
                        pr = t_pr[:, :, :, :W]
                        nc.vector.tensor_tensor(
                            out=pr,
                            in0=t_lm[:, :, :W].rearrange("p g (o w) -> p g o w", o=1).to_broadcast([P, G, 2, W]),
                            in1=rv, op=Alu.mult)
                        ft = spool.tile([P, G, 2], F32, tag="ft")
                        nc.vector.tensor_reduce(ft[:], pr, mybir.AxisListType.X, Alu.add)

                    # --- x-value select: val = x[s, feat] ---
                    t_xp = xppool.tile([P, G, INPUT_DIM], F32, tag="xprod")
                    if d == 0:
                        # fused: xprod = (iota == f0) * x
                        nc.vector.scalar_tensor_tensor(
                            out=t_xp[:],
                            in0=t_iota[:, :, :].to_broadcast([P, G, INPUT_DIM]),
                            scalar=f0, in1=t_x[:],
                            op0=Alu.is_equal, op1=Alu.mult)
                    else:
                        ftb = spool.tile([P, G, 1], mybir.dt.bfloat16, tag="ftb")
                        nc.vector.tensor_copy(out=ftb[:], in_=ft[:, :, 0:1])
                        t_xm = xmpool.tile([P, G, INPUT_DIM], mybir.dt.bfloat16,
                                           tag="xmask")
                        nc.vector.tensor_tensor(
                            out=t_xm[:],
                            in0=t_iotab[:, :, :].to_broadcast([P, G, INPUT_DIM]),
                            in1=ftb[:, :, :].to_broadcast([P, G, INPUT_DIM]),
                            op=Alu.is_equal)
                        nc.vector.tensor_tensor(out=t_xp[:], in0=t_xm[:], in1=t_x[:],
                                                op=Alu.mult)
                    val = spool.tile([P, G, 1], F32, tag="val")
                    nc.vector.tensor_reduce(val[:], t_xp[:], mybir.AxisListType.X,
                                            Alu.add)

                    # --- bit + node update ---
                    bit = spool.tile([P, G, 1], F32, tag="bit")
                    if d == 0:
                        nc.vector.tensor_scalar(
                            out=bit[:], in0=val[:], scalar1=t0, scalar2=None,
                            op0=Alu.is_gt)
                    else:
                        nc.vector.tensor_tensor(out=bit[:], in0=val[:],
                                                in1=ft[:, :, 1:2], op=Alu.is_gt)

                    if d == 0:
                        node = bit
                    elif d < 7:
                        nn = spool.tile([P, G, 1], F32, tag="node")
                        nc.vector.scalar_tensor_tensor(
                            out=nn[:], in0=node[:], scalar=2.0, in1=bit[:],
                            op0=Alu.mult, op1=Alu.add)
                        node = nn
                    elif d == 7:
                        lnode = bit
                    else:
                        ln = spool.tile([P, G, 1], F32, tag="lnode")
                        nc.vector.scalar_tensor_tensor(
                            out=ln[:], in0=lnode[:], scalar=2.0, in1=bit[:],
                            op0=Alu.mult, op1=Alu.add)
                        lnode = ln

                    if d == 6:
                        node7 = node
                        reci = spool.tile([P, G], I32, tag="reci")
                        nc.vector.tensor_copy(out=reci[:], in_=node[:])
                        t_rec = rpool.tile([P, G, REC_W], F32, tag="rec")
                        for t in range(G):
                            nc.gpsimd.indirect_dma_start(
                                out=t_rec[:, t, :], out_offset=None,
                                in_=rectab[:, :],
                                in_offset=bass.IndirectOffsetOnAxis(
                                    ap=reci[:, t:t + 1], axis=0))

                # leaf = node7 * 32 + lnode
                leaf = spool.tile([P, G, 1], F32, tag="leaf")
                nc.vector.scalar_tensor_tensor(
                    out=leaf[:], in0=node7[:], scalar=32.0, in1=lnode[:],
                    op0=Alu.mult, op1=Alu.add)
                leafi = spool.tile([P, G], I32, tag="leafi")
                nc.vector.tensor_copy(out=leafi[:], in_=leaf[:])

                t_or = opool.tile([P, G, N_CLASSES], F32, tag="orow")
                for t in range(G):
                    nc.gpsimd.indirect_dma_start(
                        out=t_or[:, t, :], out_offset=None, in_=smx[:, :],
                        in_offset=bass.IndirectOffsetOnAxis(
                            ap=leafi[:, t:t + 1], axis=0))
                nc.sync.dma_start(og_all[:, g], t_or[:])

            if rep_ctx is not None:
                rep_ctx.__exit__(None, None, None)

    nc.compile()
    return nc


def _host_tables(split_features, split_thresholds):
    feat = np.clip(np.floor(split_features), 0, INPUT_DIM - 1).astype(np.int64)
    thr = split_thresholds.astype(np.float32)
    featf = feat.astype(np.float32)

    iota = np.broadcast_to(np.arange(INPUT_DIM, dtype=np.float32),
                           (P, INPUT_DIM)).copy()
    ftlev = {}
    for d in range(1, 7):
        W = 2 ** d
        lo = W - 1
        tab = np.empty((2, W), np.float32)
        tab[0] = featf[lo:lo + W]
        tab[1] = thr[lo:lo + W]
        ftlev[d] = np.broadcast_to(tab, (P, 2, W)).copy()

    rec = np.empty((P, REC_W), np.float32)
    for l7 in range(P):
        for j in range(5):
            W = 2 ** j
            lvl_base = 2 ** (7 + j) - 1
            for l in range(W):
                n = lvl_base + l7 * W + l
                off = 2 * (W - 1 + l)
                rec[l7, off] = featf[n]
                rec[l7, off + 1] = thr[n]
    f0 = float(featf[0])
    t0 = float(thr[0])
    return iota, ftlev, rec, f0, t0


def _to_bf16(a):
    import ml_dtypes
    return a.astype(ml_dtypes.bfloat16)


_PROG_CACHE = {}


def kernel(x, split_features, split_thresholds, leaf_probabilities):
    x = np.asarray(x, dtype=np.float32)
    split_features = np.asarray(split_features, dtype=np.float32)
    split_thresholds = np.asarray(split_thresholds, dtype=np.float32)
    leaf_probabilities = np.asarray(leaf_probabilities, dtype=np.float32)

    B = x.shape[0]
    G = 24                                  # tiles per group
    per_core_min = (B + NCORES - 1) // NCORES
    tiles_pc = (per_core_min + P - 1) // P  # tiles needed per core
    NG = (tiles_pc + G - 1) // G            # groups per core
    S = P * G * NG                          # padded samples per core

    iota, ftlev, rec, f0, t0 = _host_tables(split_features, split_thresholds)

    key = (G, NG, f0, t0)
    nc = _PROG_CACHE.get(key)
    if nc is None:
        nc = _build_program(G, NG, f0, t0)
        _PROG_CACHE[key] = nc

    in_maps = []
    for c in range(NCORES):
        lo = c * S
        hi = min(lo + S, B)
        shard = np.empty((S, INPUT_DIM), np.float32)
        if hi > lo:
            shard[:hi - lo] = x[lo:hi]
            if hi - lo < S:
                shard[hi - lo:] = x[0]
        else:
            shard[:] = x[0]
        m = {"x": shard, "lp": leaf_probabilities, "iota": iota,
             "iotab": _to_bf16(iota),
             "rectab": rec}
        for d in range(1, 7):
            m[f"ft{d}"] = ftlev[d]
        in_maps.append(m)

    res = run_bass_kernel_spmd(nc, in_maps, core_ids=list(range(NCORES)))

    outs = []
    for c in range(NCORES):
        lo = c * S
        hi = min(lo + S, B)
        if hi > lo:
            outs.append(res.results[c]["out"][:hi - lo])
    return np.concatenate(outs, axis=0)



# revision 5
# speedup vs baseline: 1.1894x; 1.1894x over previous
"""Trainium2 Bass kernel for DecisionTreeModule forward (PE-matmul design).

Per 128-sample tile (samples on PSUM/SBUF partitions):
  1. PE transpose x -> xT (two 128x128 chunks).
  2. MM1 (fp32): Sel[K=f, M=127nodes] @ xT[K=f, N=s] -> cmpT PSUM [node, s]:
     the value x[s, feat_n] for ALL 127 top-tree nodes at once (one-hot
     columns make the sums exact f32 row-extractions).
  3. bits[n, s] = cmpT > thr_n (bf16); row 127 forced to 1 via thr=-1.
  4. MM2 (bf16): Wc[K=n, M=leaf] @ bits -> scoreT PSUM [leaf, s]; score==7
     exactly for the unique depth-7 leaf whose path is consistent with the
     bits. onehotT = is_equal(scoreT, 7) (f32).
  5. MM3 (fp32): onehotT[K=leaf, M=s] @ Rec[K=leaf, 64] -> rec PSUM [s, 64]:
     exact f32 extraction of the 31 (feat, thr) pairs of the sample's
     depth-7 subtree + the subtree id (col 62).
Deep levels 7-11: narrow selects from rec + 256-wide one-hot x-selects
(masks/mults on DVE, reduces on GpSimd).
Output: leaf rows fetched from a padded softmax table via dma_gather
(indices rewrapped once through DRAM into the 16-partition format).
"""
import sys
sys.path.insert(0, "/opt/trn_rl_repo")

import numpy as np
import concourse.bacc as bacc
import concourse.bass as bass
import concourse.mybir as mybir
import concourse.tile as tile
from concourse.bass_utils import run_bass_kernel_spmd

P = 128
INPUT_DIM = 256
N_CLASSES = 100
MAX_DEPTH = 12
N_NODES = 2 ** MAX_DEPTH - 1
N_LEAVES = 2 ** MAX_DEPTH
NCORES = 8
SMXW = 128                       # padded softmax row (512B)
RECW = 64                        # rec row: 31 pairs + n128 + pad

F32 = mybir.dt.float32
BF16 = mybir.dt.bfloat16
I16 = mybir.dt.int16
Alu = mybir.AluOpType
Act = mybir.ActivationFunctionType


def _build_program(G: int, NG: int, repeat: int = 1):
    C = G * NG                   # tile-columns per core
    S = P * C
    CB = next(d for d in range(8, 0, -1) if C % d == 0)  # cols per out batch
    OB = C // CB                 # out batches
    nc = bacc.Bacc("TRN2", target_bir_lowering=False, debug=False)

    x = nc.dram_tensor("x", [S, INPUT_DIM], F32, kind="ExternalInput")
    lp = nc.dram_tensor("lp", [N_LEAVES, N_CLASSES], F32, kind="ExternalInput")
    selt = nc.dram_tensor("selt", [P, 2, P], F32, kind="ExternalInput")
    thrt = nc.dram_tensor("thrt", [P, 1], F32, kind="ExternalInput")
    wcc = nc.dram_tensor("wcc", [P, P], BF16, kind="ExternalInput")
    rect = nc.dram_tensor("rect", [P, RECW], F32, kind="ExternalInput")
    ident = nc.dram_tensor("ident", [P, P], F32, kind="ExternalInput")
    iotab = nc.dram_tensor("iotab", [P, INPUT_DIM], BF16, kind="ExternalInput")
    out = nc.dram_tensor("out", [S, N_CLASSES], F32, kind="ExternalOutput")
    smx = nc.dram_tensor("smx", [N_LEAVES, SMXW], F32, kind="Internal")
    wdram = nc.dram_tensor("wdram", [16, C * 8], I16, kind="Internal")

    # sample (p, c) lives at DRAM row p*C + c  -> contiguous per partition
    xg_all = x[:, :].rearrange("(p g t) f -> p g t f", p=P, g=NG)
    og_all = out[:, :].rearrange("(p b t) k -> p b (t k)", p=P, b=OB)
    lp_r = lp[:, :].rearrange("(p c) k -> p c k", p=P)
    smx_r = smx[:, :].rearrange("(p c) k -> p c k", p=P)
    wview = wdram[:, :].rearrange("q (c e) -> q c e", e=8)

    with tile.TileContext(nc) as tc:
        with tc.tile_pool(name="cns", bufs=1) as cpool, \
             tc.tile_pool(name="xg", bufs=2) as xpool, \
             tc.tile_pool(name="pet", bufs=2) as pepool, \
             tc.tile_pool(name="rec", bufs=2) as rpool, \
             tc.tile_pool(name="mask", bufs=1) as mpool, \
             tc.tile_pool(name="prod", bufs=1) as ppool, \
             tc.tile_pool(name="sml", bufs=4) as spool, \
             tc.tile_pool(name="li", bufs=1) as lipool, \
             tc.tile_pool(name="ob", bufs=2) as opool, \
             tc.psum_pool(name="ps", bufs=2) as pspool:

            # ---- constants ----
            t_sel = cpool.tile([P, 2, P], F32)
            nc.sync.dma_start(t_sel[:], selt[:, :, :])
            t_thr = cpool.tile([P, 1], F32)
            nc.sync.dma_start(t_thr[:], thrt[:, :])
            t_wcc = cpool.tile([P, P], BF16)
            nc.sync.dma_start(t_wcc[:], wcc[:, :])
            t_rect = cpool.tile([P, RECW], F32)
            nc.sync.dma_start(t_rect[:], rect[:, :])
            t_id = cpool.tile([P, P], F32)
            nc.sync.dma_start(t_id[:], ident[:, :])
            t_iota = cpool.tile([P, 1, INPUT_DIM], BF16)
            nc.sync.dma_start(t_iota[:], iotab[:, :].rearrange("p (o f) -> p o f", o=1))

            # ---- softmax table (padded to 128 cols) ----
            with tc.tile_pool(name="p1", bufs=2) as p1pool:
                for pc in range(8):
                    t_lp = p1pool.tile([P, 4, N_CLASSES], F32, tag="lp")
                    nc.sync.dma_start(t_lp[:], lp_r[:, 4 * pc:4 * (pc + 1), :])
                    t_smx = p1pool.tile([P, 4, SMXW], F32, tag="smx")
                    nc.vector.memset(t_smx[:], 0.0)
                    nc.scalar.activation(out=t_smx[:, :, :N_CLASSES],
                                         in_=t_lp[:], func=Act.Exp)
                    t_sum = p1pool.tile([P, 4, 1], F32, tag="sum")
                    nc.vector.tensor_reduce(t_sum[:], t_smx[:, :, :N_CLASSES],
                                            mybir.AxisListType.X, Alu.add)
                    t_rcp = p1pool.tile([P, 4, 1], F32, tag="rcp")
                    nc.vector.reciprocal(t_rcp[:], t_sum[:])
                    nc.vector.tensor_tensor(
                        out=t_smx[:, :, :N_CLASSES],
                        in0=t_smx[:, :, :N_CLASSES],
                        in1=t_rcp[:, :, :].to_broadcast([P, 4, N_CLASSES]),
                        op=Alu.mult)
                    nc.sync.dma_start(smx_r[:, 4 * pc:4 * (pc + 1), :], t_smx[:])

            t_li = lipool.tile([P, C], F32)

            rep_ctx = tc.For_i(0, repeat, 1) if repeat > 1 else None
            if rep_ctx is not None:
                rep_ctx.__enter__()

            for g in range(NG):
                t_x = xpool.tile([P, G, INPUT_DIM], F32, tag="x")
                nc.sync.dma_start(t_x[:], xg_all[:, g])
                t_rec = rpool.tile([P, G, RECW], F32, tag="rec")

                for c in range(G):
                    ps_t = pspool.tile([P, P], F32, tag="pt")
                    t_xT = pepool.tile([P, 2, P], F32, tag="xT")
                    nc.tensor.transpose(ps_t[:], t_x[:, c, 0:P], t_id[:])
                    nc.scalar.activation(out=t_xT[:, 0, :], in_=ps_t[:],
                                         func=Act.Copy)
                    ps_t2 = pspool.tile([P, P], F32, tag="pt")
                    nc.tensor.transpose(ps_t2[:], t_x[:, c, P:2 * P], t_id[:])
                    nc.scalar.activation(out=t_xT[:, 1, :], in_=ps_t2[:],
                                         func=Act.Copy)

                    ps_cmp = pspool.tile([P, P], F32, tag="pc")
                    nc.tensor.matmul(ps_cmp[:], t_sel[:, 0, :], t_xT[:, 0, :],
                                     start=True, stop=False)
                    nc.tensor.matmul(ps_cmp[:], t_sel[:, 1, :], t_xT[:, 1, :],
                                     start=False, stop=True)

                    t_bits = pepool.tile([P, P], BF16, tag="bits")
                    nc.vector.tensor_tensor(
                        out=t_bits[:], in0=ps_cmp[:],
                        in1=t_thr[:, :].to_broadcast([P, P]), op=Alu.is_gt)

                    ps_sc = pspool.tile([P, P], F32, tag="psc")
                    nc.tensor.matmul(ps_sc[:], t_wcc[:, :], t_bits[:],
                                     start=True, stop=True)
                    t_oh = pepool.tile([P, P], F32, tag="oh")
                    nc.vector.tensor_scalar(out=t_oh[:], in0=ps_sc[:],
                                            scalar1=7.0, scalar2=None,
                                            op0=Alu.is_equal)
                    ps_rec = pspool.tile([P, RECW], F32, tag="pr")
                    nc.tensor.matmul(ps_rec[:], t_oh[:], t_rect[:, :],
                                     start=True, stop=True)
                    nc.scalar.activation(out=t_rec[:, c, :], in_=ps_rec[:],
                                         func=Act.Copy)

                # ---- deep levels 7..11 on the whole group ----
                lnode = None
                for j in range(5):
                    W = 2 ** j
                    base = 2 * (W - 1)
                    if j == 0:
                        ft = t_rec[:, :, 0:2]
                    else:
                        t_lm = mpool.tile([P, G, 16], BF16, tag="lmask")
                        lnb = spool.tile([P, G, 1], BF16, tag="lnb")
                        nc.vector.tensor_copy(out=lnb[:], in_=lnode[:])
                        nc.vector.tensor_tensor(
                            out=t_lm[:, :, :W],
                            in0=t_iota[:, :, :W].to_broadcast([P, G, W]),
                            in1=lnb[:, :, :].to_broadcast([P, G, W]),
                            op=Alu.is_equal)
                        rv = t_rec[:, :, base:base + 2 * W].rearrange(
                            "p g (l c) -> p g c l", c=2)
                        t_pr = ppool.tile([P, G, 2, 16], F32, tag="lprod")
                        nc.vector.tensor_tensor(
                            out=t_pr[:, :, :, :W],
                            in0=t_lm[:, :, :W].rearrange(
                                "p g (o w) -> p g o w", o=1).to_broadcast([P, G, 2, W]),
                            in1=rv, op=Alu.mult)
                        ft = spool.tile([P, G, 2], F32, tag="ft")
                        nc.vector.tensor_reduce(ft[:], t_pr[:, :, :, :W],
                                                mybir.AxisListType.X, Alu.add)

                    ftb = spool.tile([P, G, 1], BF16, tag="ftb")
                    nc.vector.tensor_copy(out=ftb[:], in_=ft[:, :, 0:1])
                    t_xm = mpool.tile([P, G, INPUT_DIM], BF16, tag="xmask")
                    nc.vector.tensor_tensor(
                        out=t_xm[:],
                        in0=t_iota[:, :, :].to_broadcast([P, G, INPUT_DIM]),
                        in1=ftb[:, :, :].to_broadcast([P, G, INPUT_DIM]),
                        op=Alu.is_equal)
                    t_xp = ppool.tile([P, G, INPUT_DIM], F32, tag="xprod")
                    mul_eng = nc.gpsimd if j in (1, 3) else nc.vector
                    mul_eng.tensor_tensor(out=t_xp[:], in0=t_xm[:], in1=t_x[:],
                                          op=Alu.mult)
                    val = spool.tile([P, G, 1], F32, tag="val")
                    nc.vector.tensor_reduce(val[:], t_xp[:],
                                            mybir.AxisListType.X, Alu.add)
                    bit = spool.tile([P, G, 1], F32, tag="bit")
                    nc.vector.tensor_tensor(out=bit[:], in0=val[:],
                                            in1=ft[:, :, 1:2], op=Alu.is_gt)
                    if j == 0:
                        lnode = bit
                    else:
                        ln = spool.tile([P, G, 1], F32, tag="lnode")
                        nc.vector.scalar_tensor_tensor(
                            out=ln[:], in0=lnode[:], scalar=2.0, in1=bit[:],
                            op0=Alu.mult, op1=Alu.add)
                        lnode = ln

                # leaf row = n128*32 + lnode
                nc.vector.scalar_tensor_tensor(
                    out=t_li[:, g * G:(g + 1) * G],
                    in0=t_rec[:, :, 62], scalar=32.0, in1=lnode[:, :, 0],
                    op0=Alu.mult, op1=Alu.add)

            # ---- output phase ----
            t_li16 = lipool.tile([P, C], I16)
            nc.vector.tensor_copy(out=t_li16[:], in_=t_li[:])
            for k in range(8):
                nc.sync.dma_start(wview[:, :, k], t_li16[16 * k:16 * (k + 1), :])
            t_w = lipool.tile([P, C * 8], I16)
            for cc in range(8):
                nc.sync.dma_start(t_w[16 * cc:16 * (cc + 1), :], wdram[:, :])

            for b in range(OB):
                t_ob = opool.tile([P, CB, SMXW], F32, tag="ob")
                nc.gpsimd.dma_gather(
                    out_ap=t_ob[:],
                    in_ap=smx[:, :],
                    idxs_ap=t_w[:, b * (CB * 8):(b + 1) * (CB * 8)],
                    num_idxs=CB * P,
                    num_idxs_reg=CB * P,
                    elem_size=SMXW)
                t_oc = opool.tile([P, CB * N_CLASSES], F32, tag="oc")
                nc.scalar.activation(
                    out=t_oc[:].rearrange("p (t k) -> p t k", k=N_CLASSES),
                    in_=t_ob[:, :, :N_CLASSES], func=Act.Copy)
                nc.sync.dma_start(og_all[:, b], t_oc[:])

            if rep_ctx is not None:
                rep_ctx.__exit__(None, None, None)

    nc.compile()
    return nc


def _host_tables(split_features, split_thresholds):
    feat = np.clip(np.floor(split_features), 0, INPUT_DIM - 1).astype(np.int64)
    thr = split_thresholds.astype(np.float32)

    selt = np.zeros((P, 2, P), np.float32)
    for n in range(127):
        f = feat[n]
        selt[f % P, f // P, n] = 1.0
    thrt = np.full((P, 1), -1.0, np.float32)
    thrt[:127, 0] = thr[:127]

    wcc = np.zeros((P, P), np.float32)
    for l in range(128):
        node = 0
        nz = 0
        for d in range(7):
            b = (l >> (6 - d)) & 1
            wcc[node, l] = 1.0 if b else -1.0
            if not b:
                nz += 1
            node = 2 * node + 1 + b
        wcc[127, l] = float(nz)

    rect = np.zeros((P, RECW), np.float32)
    for l in range(128):
        for j in range(5):
            W = 2 ** j
            lvl_base = 2 ** (7 + j) - 1
            for ll in range(W):
                n = lvl_base + l * W + ll
                off = 2 * (W - 1 + ll)
                rect[l, off] = float(feat[n])
                rect[l, off + 1] = thr[n]
        rect[l, 62] = float(l)

    ident = np.eye(P, dtype=np.float32)
    iota = np.broadcast_to(np.arange(INPUT_DIM, dtype=np.float32),
                           (P, INPUT_DIM)).copy()
    return selt, thrt, wcc, rect, ident, iota


def _to_bf16(a):
    import ml_dtypes
    return np.asarray(a, dtype=np.float32).astype(ml_dtypes.bfloat16)


_PROG_CACHE = {}


def kernel(x, split_features, split_thresholds, leaf_probabilities):
    x = np.asarray(x, dtype=np.float32)
    split_features = np.asarray(split_features, dtype=np.float32)
    split_thresholds = np.asarray(split_thresholds, dtype=np.float32)
    leaf_probabilities = np.asarray(leaf_probabilities, dtype=np.float32)

    B = x.shape[0]
    G, NG = 24, 21
    C = G * NG
    S = P * C
    assert S * NCORES >= B

    selt, thrt, wcc, rect, ident, iota = _host_tables(
        split_features, split_thresholds)

    key = (G, NG)
    nc = _PROG_CACHE.get(key)
    if nc is None:
        nc = _build_program(G, NG)
        _PROG_CACHE[key] = nc

    in_maps = []
    for c in range(NCORES):
        lo = c * S
        hi = min(lo + S, B)
        shard = np.empty((S, INPUT_DIM), np.float32)
        if hi > lo:
            shard[:hi - lo] = x[lo:hi]
            if hi - lo < S:
                shard[hi - lo:] = x[0]
        else:
            shard[:] = x[0]
        m = {"x": shard, "lp": leaf_probabilities, "selt": selt, "thrt": thrt,
             "wcc": _to_bf16(wcc), "rect": rect, "ident": ident,
             "iotab": _to_bf16(iota)}
        in_maps.append(m)

    res = run_bass_kernel_spmd(nc, in_maps, core_ids=list(range(NCORES)))

    outs = []
    for c in range(NCORES):
        lo = c * S
        hi = min(lo + S, B)
        if hi > lo:
            outs.append(res.results[c]["out"][:hi - lo])
    return np.concatenate(outs, axis=0)


# revision 6
# speedup vs baseline: 1.5871x; 1.3344x over previous
"""Trainium2 Bass kernel for DecisionTreeModule forward (PE-matmul design).

Per 128-sample tile (samples on PSUM/SBUF partitions):
  1. PE transpose x -> xT (two 128x128 chunks).
  2. MM1 (fp32): Sel[K=f, M=127nodes] @ xT[K=f, N=s] -> cmpT PSUM [node, s]:
     the value x[s, feat_n] for ALL 127 top-tree nodes at once (one-hot
     columns make the sums exact f32 row-extractions).
  3. bits[n, s] = cmpT > thr_n (bf16); row 127 forced to 1 via thr=-1.
  4. MM2 (bf16): Wc[K=n, M=leaf] @ bits -> scoreT PSUM [leaf, s]; score==7
     exactly for the unique depth-7 leaf whose path is consistent with the
     bits. onehotT = is_equal(scoreT, 7) (f32).
  5. MM3 (fp32): onehotT[K=leaf, M=s] @ Rec[K=leaf, 64] -> rec PSUM [s, 64]:
     exact f32 extraction of the 31 (feat, thr) pairs of the sample's
     depth-7 subtree + the subtree id (col 62).
Deep levels 7-11: narrow selects from rec + 256-wide one-hot x-selects
(masks/mults on DVE, reduces on GpSimd).
Output: leaf rows fetched from a padded softmax table via dma_gather
(indices rewrapped once through DRAM into the 16-partition format).
"""
import os
import sys
sys.path.insert(0, "/opt/trn_rl_repo")
ABL_DEEP = bool(os.environ.get('ABL_DEEP'))
ABL_TOP = bool(os.environ.get('ABL_TOP'))

import numpy as np
import concourse.bacc as bacc
import concourse.bass as bass
import concourse.mybir as mybir
import concourse.tile as tile
from concourse.bass_utils import run_bass_kernel_spmd

P = 128
INPUT_DIM = 256
N_CLASSES = 100
MAX_DEPTH = 12
N_NODES = 2 ** MAX_DEPTH - 1
N_LEAVES = 2 ** MAX_DEPTH
NCORES = 8
SMXW = 128                       # padded softmax row (512B)
RECW = 64                        # rec row: 31 pairs + n128 + pad

F32 = mybir.dt.float32
BF16 = mybir.dt.bfloat16
I16 = mybir.dt.int16
Alu = mybir.AluOpType
Act = mybir.ActivationFunctionType


def _build_program(G: int, NG: int, repeat: int = 1):
    C = G * NG                   # tile-columns per core
    S = P * C
    CB = next(d for d in range(8, 0, -1) if C % d == 0)  # cols per out batch
    OB = C // CB                 # out batches
    nc = bacc.Bacc("TRN2", target_bir_lowering=False, debug=False)

    x = nc.dram_tensor("x", [S, INPUT_DIM], F32, kind="ExternalInput")
    lp = nc.dram_tensor("lp", [N_LEAVES, N_CLASSES], F32, kind="ExternalInput")
    selt = nc.dram_tensor("selt", [P, 2, P], F32, kind="ExternalInput")
    thrt = nc.dram_tensor("thrt", [P, 1], F32, kind="ExternalInput")
    wcc = nc.dram_tensor("wcc", [P, P], BF16, kind="ExternalInput")
    rect = nc.dram_tensor("rect", [P, RECW], F32, kind="ExternalInput")
    ident = nc.dram_tensor("ident", [P, P], F32, kind="ExternalInput")
    iotab = nc.dram_tensor("iotab", [P, INPUT_DIM], BF16, kind="ExternalInput")
    out = nc.dram_tensor("out", [S, N_CLASSES], F32, kind="ExternalOutput")
    smx = nc.dram_tensor("smx", [N_LEAVES, SMXW], F32, kind="Internal")
    wdram = nc.dram_tensor("wdram", [16, C * 8], I16, kind="Internal")

    # sample (p, c) lives at DRAM row p*C + c  -> contiguous per partition
    xg_all = x[:, :].rearrange("(p g t) f -> p g t f", p=P, g=NG)
    og_all = out[:, :].rearrange("(p b t) k -> p b (t k)", p=P, b=OB)
    lp_r = lp[:, :].rearrange("(p c) k -> p c k", p=P)
    smx_r = smx[:, :].rearrange("(p c) k -> p c k", p=P)
    wview = wdram[:, :].rearrange("q (c e) -> q c e", e=8)

    with tile.TileContext(nc) as tc:
        with tc.tile_pool(name="cns", bufs=1) as cpool, \
             tc.tile_pool(name="xg", bufs=2) as xpool, \
             tc.tile_pool(name="pet", bufs=4) as pepool, \
             tc.tile_pool(name="rec", bufs=2) as rpool, \
             tc.tile_pool(name="mask", bufs=2) as mpool, \
             tc.tile_pool(name="prod", bufs=2) as ppool, \
             tc.tile_pool(name="sml", bufs=4) as spool, \
             tc.tile_pool(name="li", bufs=1) as lipool, \
             tc.tile_pool(name="ob", bufs=2) as opool, \
             tc.psum_pool(name="ps", bufs=2) as pspool:

            # ---- constants ----
            t_sel = cpool.tile([P, 2, P], F32)
            nc.sync.dma_start(t_sel[:], selt[:, :, :])
            t_thr = cpool.tile([P, 1], F32)
            nc.sync.dma_start(t_thr[:], thrt[:, :])
            t_wcc = cpool.tile([P, P], BF16)
            nc.sync.dma_start(t_wcc[:], wcc[:, :])
            t_rect = cpool.tile([P, RECW], F32)
            nc.sync.dma_start(t_rect[:], rect[:, :])
            t_id = cpool.tile([P, P], F32)
            nc.sync.dma_start(t_id[:], ident[:, :])
            t_iota = cpool.tile([P, 1, INPUT_DIM], BF16)
            nc.sync.dma_start(t_iota[:], iotab[:, :].rearrange("p (o f) -> p o f", o=1))

            # ---- softmax table (padded to 128 cols) ----
            with tc.tile_pool(name="p1", bufs=2) as p1pool:
                for pc in range(8):
                    t_lp = p1pool.tile([P, 4, N_CLASSES], F32, tag="lp")
                    nc.sync.dma_start(t_lp[:], lp_r[:, 4 * pc:4 * (pc + 1), :])
                    t_smx = p1pool.tile([P, 4, SMXW], F32, tag="smx")
                    nc.vector.memset(t_smx[:], 0.0)
                    nc.scalar.activation(out=t_smx[:, :, :N_CLASSES],
                                         in_=t_lp[:], func=Act.Exp)
                    t_sum = p1pool.tile([P, 4, 1], F32, tag="sum")
                    nc.vector.tensor_reduce(t_sum[:], t_smx[:, :, :N_CLASSES],
                                            mybir.AxisListType.X, Alu.add)
                    t_rcp = p1pool.tile([P, 4, 1], F32, tag="rcp")
                    nc.vector.reciprocal(t_rcp[:], t_sum[:])
                    nc.vector.tensor_tensor(
                        out=t_smx[:, :, :N_CLASSES],
                        in0=t_smx[:, :, :N_CLASSES],
                        in1=t_rcp[:, :, :].to_broadcast([P, 4, N_CLASSES]),
                        op=Alu.mult)
                    nc.sync.dma_start(smx_r[:, 4 * pc:4 * (pc + 1), :], t_smx[:])

            t_li = lipool.tile([P, C], F32)

            rep_ctx = tc.For_i(0, repeat, 1) if repeat > 1 else None
            if rep_ctx is not None:
                rep_ctx.__enter__()

            for g in range(NG):
                t_x = xpool.tile([P, G, INPUT_DIM], F32, tag="x")
                nc.sync.dma_start(t_x[:], xg_all[:, g])
                t_rec = rpool.tile([P, G, RECW], F32, tag="rec")

                if ABL_TOP:
                    nc.vector.memset(t_rec[:], 1.0)
                for c in ([] if ABL_TOP else range(G)):
                    ps_t = pspool.tile([P, P], F32, tag="pt")
                    t_xT = pepool.tile([P, 2, P], F32, tag="xT")
                    nc.tensor.transpose(ps_t[:], t_x[:, c, 0:P], t_id[:])
                    nc.scalar.activation(out=t_xT[:, 0, :], in_=ps_t[:],
                                         func=Act.Copy)
                    ps_t2 = pspool.tile([P, P], F32, tag="pt")
                    nc.tensor.transpose(ps_t2[:], t_x[:, c, P:2 * P], t_id[:])
                    nc.scalar.activation(out=t_xT[:, 1, :], in_=ps_t2[:],
                                         func=Act.Copy)

                    ps_cmp = pspool.tile([P, P], F32, tag="pc")
                    nc.tensor.matmul(ps_cmp[:], t_sel[:, 0, :], t_xT[:, 0, :],
                                     start=True, stop=False)
                    nc.tensor.matmul(ps_cmp[:], t_sel[:, 1, :], t_xT[:, 1, :],
                                     start=False, stop=True)

                    t_bits = pepool.tile([P, P], BF16, tag="bits")
                    nc.vector.tensor_tensor(
                        out=t_bits[:], in0=ps_cmp[:],
                        in1=t_thr[:, :].to_broadcast([P, P]), op=Alu.is_gt)

                    ps_sc = pspool.tile([P, P], F32, tag="psc")
                    nc.tensor.matmul(ps_sc[:], t_wcc[:, :], t_bits[:],
                                     start=True, stop=True)
                    t_oh = pepool.tile([P, P], F32, tag="oh")
                    nc.vector.tensor_scalar(out=t_oh[:], in0=ps_sc[:],
                                            scalar1=7.0, scalar2=None,
                                            op0=Alu.is_equal)
                    ps_rec = pspool.tile([P, RECW], F32, tag="pr")
                    nc.tensor.matmul(ps_rec[:], t_oh[:], t_rect[:, :],
                                     start=True, stop=True)
                    nc.scalar.activation(out=t_rec[:, c, :], in_=ps_rec[:],
                                         func=Act.Copy)

                # ---- deep levels 7..11 on the whole group ----
                lnode = None
                for j in ([] if ABL_DEEP else range(5)):
                    W = 2 ** j
                    base = 2 * (W - 1)
                    if j == 0:
                        ft = t_rec[:, :, 0:2]
                    else:
                        t_lm = mpool.tile([P, G, 16], BF16, tag="lmask")
                        lnb = spool.tile([P, G, 1], BF16, tag="lnb")
                        nc.vector.tensor_copy(out=lnb[:], in_=lnode[:])
                        nc.vector.tensor_tensor(
                            out=t_lm[:, :, :W],
                            in0=t_iota[:, :, :W].to_broadcast([P, G, W]),
                            in1=lnb[:, :, :].to_broadcast([P, G, W]),
                            op=Alu.is_equal)
                        rv = t_rec[:, :, base:base + 2 * W].rearrange(
                            "p g (l c) -> p g c l", c=2)
                        t_pr = ppool.tile([P, G, 2, 16], F32, tag="lprod")
                        nc.vector.tensor_tensor(
                            out=t_pr[:, :, :, :W],
                            in0=t_lm[:, :, :W].rearrange(
                                "p g (o w) -> p g o w", o=1).to_broadcast([P, G, 2, W]),
                            in1=rv, op=Alu.mult)
                        ft = spool.tile([P, G, 2], F32, tag="ft")
                        nc.vector.tensor_reduce(ft[:], t_pr[:, :, :, :W],
                                                mybir.AxisListType.X, Alu.add)

                    ftb = spool.tile([P, G, 1], BF16, tag="ftb")
                    nc.vector.tensor_copy(out=ftb[:], in_=ft[:, :, 0:1])
                    t_xm = mpool.tile([P, G, INPUT_DIM], BF16, tag="xmask")
                    nc.vector.tensor_tensor(
                        out=t_xm[:],
                        in0=t_iota[:, :, :].to_broadcast([P, G, INPUT_DIM]),
                        in1=ftb[:, :, :].to_broadcast([P, G, INPUT_DIM]),
                        op=Alu.is_equal)
                    t_xp = ppool.tile([P, G, INPUT_DIM], F32, tag="xprod")
                    mul_eng = nc.gpsimd if j in (0, 1, 3) else nc.vector
                    mul_eng.tensor_tensor(out=t_xp[:], in0=t_xm[:], in1=t_x[:],
                                          op=Alu.mult)
                    val = spool.tile([P, G, 1], F32, tag="val")
                    nc.vector.tensor_reduce(val[:], t_xp[:],
                                            mybir.AxisListType.X, Alu.add)
                    bit = spool.tile([P, G, 1], F32, tag="bit")
                    nc.vector.tensor_tensor(out=bit[:], in0=val[:],
                                            in1=ft[:, :, 1:2], op=Alu.is_gt)
                    if j == 0:
                        lnode = bit
                    else:
                        ln = spool.tile([P, G, 1], F32, tag="lnode")
                        nc.vector.scalar_tensor_tensor(
                            out=ln[:], in0=lnode[:], scalar=2.0, in1=bit[:],
                            op0=Alu.mult, op1=Alu.add)
                        lnode = ln

                # leaf row = n128*32 + lnode
                if ABL_DEEP:
                    nc.vector.tensor_scalar(
                        out=t_li[:, g * G:(g + 1) * G], in0=t_rec[:, :, 62],
                        scalar1=32.0, scalar2=None, op0=Alu.mult)
                else:
                    nc.vector.scalar_tensor_tensor(
                        out=t_li[:, g * G:(g + 1) * G],
                        in0=t_rec[:, :, 62], scalar=32.0, in1=lnode[:, :, 0],
                        op0=Alu.mult, op1=Alu.add)

            # ---- output phase ----
            t_li16 = lipool.tile([P, C], I16)
            nc.vector.tensor_copy(out=t_li16[:], in_=t_li[:])
            for k in range(8):
                nc.sync.dma_start(wview[:, :, k], t_li16[16 * k:16 * (k + 1), :])
            t_w = lipool.tile([P, C * 8], I16)
            for cc in range(8):
                nc.sync.dma_start(t_w[16 * cc:16 * (cc + 1), :], wdram[:, :])

            for b in range(OB):
                t_ob = opool.tile([P, CB, SMXW], F32, tag="ob")
                nc.gpsimd.dma_gather(
                    out_ap=t_ob[:],
                    in_ap=smx[:, :],
                    idxs_ap=t_w[:, b * (CB * 8):(b + 1) * (CB * 8)],
                    num_idxs=CB * P,
                    num_idxs_reg=CB * P,
                    elem_size=SMXW)
                t_oc = opool.tile([P, CB * N_CLASSES], F32, tag="oc")
                nc.scalar.activation(
                    out=t_oc[:].rearrange("p (t k) -> p t k", k=N_CLASSES),
                    in_=t_ob[:, :, :N_CLASSES], func=Act.Copy)
                nc.sync.dma_start(og_all[:, b], t_oc[:])

            if rep_ctx is not None:
                rep_ctx.__exit__(None, None, None)

    nc.compile()
    return nc


def _host_tables(split_features, split_thresholds):
    feat = np.clip(np.floor(split_features), 0, INPUT_DIM - 1).astype(np.int64)
    thr = split_thresholds.astype(np.float32)

    selt = np.zeros((P, 2, P), np.float32)
    for n in range(127):
        f = feat[n]
        selt[f % P, f // P, n] = 1.0
    thrt = np.full((P, 1), -1.0, np.float32)
    thrt[:127, 0] = thr[:127]

    wcc = np.zeros((P, P), np.float32)
    for l in range(128):
        node = 0
        nz = 0
        for d in range(7):
            b = (l >> (6 - d)) & 1
            wcc[node, l] = 1.0 if b else -1.0
            if not b:
                nz += 1
            node = 2 * node + 1 + b
        wcc[127, l] = float(nz)

    rect = np.zeros((P, RECW), np.float32)
    for l in range(128):
        for j in range(5):
            W = 2 ** j
            lvl_base = 2 ** (7 + j) - 1
            for ll in range(W):
                n = lvl_base + l * W + ll
                off = 2 * (W - 1 + ll)
                rect[l, off] = float(feat[n])
                rect[l, off + 1] = thr[n]
        rect[l, 62] = float(l)

    ident = np.eye(P, dtype=np.float32)
    iota = np.broadcast_to(np.arange(INPUT_DIM, dtype=np.float32),
                           (P, INPUT_DIM)).copy()
    return selt, thrt, wcc, rect, ident, iota


def _to_bf16(a):
    import ml_dtypes
    return np.asarray(a, dtype=np.float32).astype(ml_dtypes.bfloat16)


_PROG_CACHE = {}


def kernel(x, split_features, split_thresholds, leaf_probabilities):
    x = np.asarray(x, dtype=np.float32)
    split_features = np.asarray(split_features, dtype=np.float32)
    split_thresholds = np.asarray(split_thresholds, dtype=np.float32)
    leaf_probabilities = np.asarray(leaf_probabilities, dtype=np.float32)

    B = x.shape[0]
    G, NG = 24, 21
    C = G * NG
    S = P * C
    assert S * NCORES >= B

    selt, thrt, wcc, rect, ident, iota = _host_tables(
        split_features, split_thresholds)

    key = (G, NG)
    nc = _PROG_CACHE.get(key)
    if nc is None:
        nc = _build_program(G, NG)
        _PROG_CACHE[key] = nc

    in_maps = []
    for c in range(NCORES):
        lo = c * S
        hi = min(lo + S, B)
        shard = np.empty((S, INPUT_DIM), np.float32)
        if hi > lo:
            shard[:hi - lo] = x[lo:hi]
            if hi - lo < S:
                shard[hi - lo:] = x[0]
        else:
            shard[:] = x[0]
        m = {"x": shard, "lp": leaf_probabilities, "selt": selt, "thrt": thrt,
             "wcc": _to_bf16(wcc), "rect": rect, "ident": ident,
             "iotab": _to_bf16(iota)}
        in_maps.append(m)

    res = run_bass_kernel_spmd(nc, in_maps, core_ids=list(range(NCORES)))

    outs = []
    for c in range(NCORES):
        lo = c * S
        hi = min(lo + S, B)
        if hi > lo:
            outs.append(res.results[c]["out"][:hi - lo])
    return np.concatenate(outs, axis=0)
